# revision 29
# baseline (speedup 1.0000x reference)
"""GCNConv (out = segment_sum(val * (X@W)[col], row)) on 8 TRN2 NeuronCores.

Sharding: output rows (nodes) are sharded across the 8 cores (12500 rows
each); W is replicated.  Each core computes its shard of XW = X @ W, the
shards are AllGathered into a full XW table in every core's DRAM, and each
core then aggregates only its own output rows:

    out[r] = sum over edges (r, c) of  val * XW[c]

The aggregation is implemented as dma_gather of XW rows (the source nodes of
the core's edges, pre-sorted on the host by destination window / source
block) followed by one-hot-matrix matmuls accumulating 128-destination-row
windows in PSUM:  out_win += S @ G  where S[d, e] = val_e * [dest_e == d]
is built on the vector engine from a single fused tensor_scalar
(iota == dest) * val, and G holds the gathered XW rows (one edge per
partition).

Host-side preprocessing (inside kernel()) only shards / sorts / pads the
edge list with numpy; all FLOPs and all memory-heavy work run on device.
"""

from contextlib import ExitStack

import numpy as np

import concourse.bass as bass
import concourse.mybir as mybir
from concourse import bacc, tile
from concourse.bass_utils import run_bass_kernel_spmd

F32 = mybir.dt.float32
I16 = mybir.dt.int16
I32 = mybir.dt.int32


class Cfg:
    def __init__(self, n_nodes=100000, in_dim=256, out_dim=128, ncores=8,
                 win=128, grp=4, blk=32768):
        self.N = n_nodes
        self.IN = in_dim
        self.OUT = out_dim
        self.P = ncores
        self.R = n_nodes // ncores          # rows (nodes) per core
        self.WIN = win                      # destination window (PSUM partitions)
        self.GRP = grp                      # windows per gather group
        self.BLK = blk                      # gather-table block (int16 index limit)
        self.NW = -(-self.R // win)         # windows per core
        self.NG = -(-self.NW // grp)        # groups per core
        # The XW table is AllGathered in two halves (so block-0/1 gathers can
        # start while the second AllGather is in flight).  Table layout is
        # "half-major": half h holds rows (p, r) for r in [h*R/2, (h+1)*R/2)
        # of every rank p, concatenated by rank.
        self.N2 = self.N // 2               # rows per half
        self.R2 = self.R // 2
        self.NBH = -(-self.N2 // blk)       # blocks per half
        self.NBLK = 2 * self.NBH
        assert n_nodes % ncores == 0 and self.R % 2 == 0
        assert blk <= 32768

    def remap(self, col):
        """Node id -> position in the half-major AllGather table layout."""
        p, r = np.divmod(col, self.R)
        lo = r < self.R2
        return np.where(lo, p * self.R2 + r,
                        self.N2 + p * self.R2 + (r - self.R2))


CFG = Cfg()


def _plan(cfg, edge_row, edge_col, edge_val):
    """Partition/sort/pad the edge list per core. Returns (static, per_core).

    Static structure (identical for all cores, required for SPMD):
      - MAXS[b]: padded slot count of each (group, block) gather cell
      - instance list: (group, window-in-group, block, chunk) matmul chunks
    Per core:
      - IDX  [128, TOTS//16] int16: gather indices (16-part wrap, replicated x8)
      - DEST [128, NINST] f32: per-chunk-instance local dest row (-1 = inactive)
      - VAL  [128, NINST] f32: per-chunk-instance edge weight (0 = inactive)
    """
    P, R, WIN, GRP, BLK, NBLK = cfg.P, cfg.R, cfg.WIN, cfg.GRP, cfg.BLK, cfg.NBLK
    NW, NG = cfg.NW, cfg.NG
    NCELL = NG * NBLK

    cores = []
    for p in range(P):
        s = np.searchsorted(edge_row, p * R, side="left")
        e = np.searchsorted(edge_row, (p + 1) * R, side="left")
        r = edge_row[s:e].astype(np.int64) - p * R
        c = edge_col[s:e].astype(np.int64)
        v = edge_val[s:e].astype(np.float32)
        w = r // WIN
        g = w // GRP
        pos = cfg.remap(c)                 # position in half-major table
        half = pos // cfg.N2
        off = pos - half * cfg.N2
        b = half * cfg.NBH + off // BLK
        c = off % BLK                      # index within block
        order = np.lexsort((w, b, g))
        r, c, v, w, g, b = (a[order] for a in (r, c, v, w, g, b))
        cell = g * NBLK + b
        counts = np.bincount(cell, minlength=NCELL)
        cstart = np.concatenate([[0], np.cumsum(counts)[:-1]])
        pos = np.arange(len(r)) - cstart[cell]
        j = w - g * GRP
        cnt_cwj = np.bincount(cell * GRP + j, minlength=NCELL * GRP)
        cnt_cwj = cnt_cwj.reshape(NCELL, GRP)
        cores.append(dict(r=r, c=c, v=v, w=w, g=g, b=b, cell=cell, pos=pos,
                          counts=counts, cnt_cwj=cnt_cwj))

    all_counts = np.stack([cc["counts"] for cc in cores])      # [P, NCELL]
    per_gb = all_counts.reshape(P, NG, NBLK)
    maxs = per_gb.max(axis=(0, 1))                             # [NBLK]
    MAXS = np.maximum(128, ((maxs + 127) // 128) * 128).astype(np.int64)
    S_TOT = int(MAXS.sum())
    boff = np.concatenate([[0], np.cumsum(MAXS)[:-1]]).astype(np.int64)
    TOTS = NG * S_TOT

    # instance enumeration (static): for each (g, j): the (b, chunk) matmuls
    inst_list = []
    win_insts = {}
    maxch = int(MAXS.max()) // 128
    L = -np.ones((NCELL, maxch, GRP), np.int64)
    for g in range(NG):
        jmax = min(GRP, NW - g * GRP)
        for j in range(jmax):
            lst = []
            for b in range(NBLK):
                cell = g * NBLK + b
                c0, c1 = 10 ** 9, -1
                for cc in cores:
                    cnts = cc["cnt_cwj"][cell]
                    st = int(cnts[:j].sum())
                    en = st + int(cnts[j])
                    if en > st:
                        c0 = min(c0, st // 128)
                        c1 = max(c1, -(-en // 128))
                if c1 < 0:
                    continue
                for ch in range(c0, c1):
                    inst_id = len(inst_list)
                    inst_list.append((g, j, b, ch))
                    L[cell, ch, j] = inst_id
                    lst.append((b, ch, inst_id))
            win_insts[(g, j)] = lst
    NINST = len(inst_list)

    per_core = []
    for cc in cores:
        dest = np.full((128, max(NINST, 1)), -1.0, np.float32)
        val = np.zeros((128, max(NINST, 1)), np.float32)
        # -1 = "skip" (no DMA); real indices first in each cell, -1 tail.
        idx = np.full(TOTS, -1, np.int16)
        slot = cc["g"] * S_TOT + boff[cc["b"]] + cc["pos"]
        idx[slot] = cc["c"].astype(np.int16)   # already block-local
        # per-cell live count for num_idxs_reg; keep a >=16 floor (the gather
        # ucode wraps indices over 16 channels) by padding tiny cells with
        # real row-0 reads.
        nreal = cc["counts"].astype(np.int64).copy()
        for cell_id in np.nonzero(nreal < 16)[0]:
            g_, b_ = divmod(cell_id, NBLK)
            base = g_ * S_TOT + boff[b_]
            idx[base + nreal[cell_id]:base + 16] = 0
            nreal[cell_id] = 16
        chunk = cc["pos"] // 128
        jj = cc["w"] - cc["g"] * GRP
        inst = L[cc["cell"], chunk, jj]
        assert (inst >= 0).all()
        part = cc["pos"] % 128
        dest[part, inst] = (cc["r"] % WIN).astype(np.float32)
        val[part, inst] = cc["v"]
        idx128 = np.tile(idx.reshape(-1, 16).T, (8, 1))        # [128, TOTS//16]
        per_core.append(dict(idx=np.ascontiguousarray(idx128),
                             dest=dest, val=val,
                             nreal=nreal.astype(np.int32).reshape(1, -1)))

    static = dict(MAXS=MAXS, S_TOT=S_TOT, TOTS=TOTS, boff=boff,
                  NINST=max(NINST, 1), win_insts=win_insts)
    return static, per_core


def _build(cfg, static, single_core=False, xw_mode="ag", use_gather=True):
    """Trace + schedule + compile the SPMD Bass program (one NEFF, 8 cores).

    single_core=True builds a collective-free variant for TimelineSim cost
    modeling: the gather table is an ExternalInput instead of the AllGather
    output (the AllGather itself costs ~35us extra; see collectives.md).

    xw_mode: "ag" (shard + AllGather), "fill" (no collective; xw_full filled
    with 8 DMA copies of the local shard -- wrong data, crash-bisect only),
    "local" (AllGather with Local instead of Shared scratchpad).
    use_gather=False replaces dma_gather with contiguous DMA reads of the
    same size (wrong data, crash-bisect only).
    """
    R, IN, OUT, WIN, GRP, BLK, NBLK = (cfg.R, cfg.IN, cfg.OUT, cfg.WIN,
                                       cfg.GRP, cfg.BLK, cfg.NBLK)
    NW, NG, N = cfg.NW, cfg.NG, cfg.N
    MAXS, S_TOT, TOTS = static["MAXS"], static["S_TOT"], static["TOTS"]
    boff, NINST, win_insts = static["boff"], static["NINST"], static["win_insts"]
    NCHUNK = [int(m) // 128 for m in MAXS]

    N2, R2, NBH = cfg.N2, cfg.R2, cfg.NBH

    nc = bacc.Bacc("TRN2", target_bir_lowering=False, debug=False,
                   num_devices=1 if single_core else cfg.P)
    XWFULLd = None
    if single_core:
        XWFULLd = nc.dram_tensor("XWFULL", [N, OUT], F32,
                                 kind="ExternalInput").ap()
    XTd = nc.dram_tensor("XT", [IN, R], F32, kind="ExternalInput").ap()
    Wd = nc.dram_tensor("W", [IN, OUT], F32, kind="ExternalInput").ap()
    IDXd = nc.dram_tensor("IDX", [128, TOTS // 16], I16, kind="ExternalInput").ap()
    DESTd = nc.dram_tensor("DEST", [128, NINST], F32, kind="ExternalInput").ap()
    VALd = nc.dram_tensor("VAL", [128, NINST], F32, kind="ExternalInput").ap()
    NCELL = NG * NBLK
    NREALd = nc.dram_tensor("NREAL", [1, NCELL], I32, kind="ExternalInput").ap()
    OUTd = nc.dram_tensor("OUT", [R, OUT], F32, kind="ExternalOutput").ap()

    blk_rows = [min(BLK, N2 - (b % NBH) * BLK) for b in range(NBLK)]

    with tile.TileContext(nc) as tc:
        with (
            ExitStack() as stack,
            tc.tile_pool(name="dram", bufs=1, space="DRAM") as dram,
            tc.tile_pool(name="consts", bufs=1) as consts,
            tc.tile_pool(name="xtp", bufs=2) as xtp,
            tc.tile_pool(name="xwstage", bufs=3) as xwstage,
            tc.tile_pool(name="gpool", bufs=2) as gpool,
            tc.tile_pool(name="stp", bufs=4) as stp,
            tc.tile_pool(name="outp", bufs=4) as outp,
            tc.tile_pool(name="psum_xw", bufs=2, space="PSUM") as psum_xw,
            tc.tile_pool(name="psum_e", bufs=4, space="PSUM") as psum_e,
        ):
            xw_lo0 = dram.tile([R2, OUT], F32)
            xw_lo1 = dram.tile([R2, OUT], F32)
            if single_core:
                xw_half = [XWFULLd[0:N2, :], XWFULLd[N2:N, :]]
            else:
                aspace = "Shared" if xw_mode == "ag" else "Local"
                xw_h0 = dram.tile([N2, OUT], F32, addr_space=aspace)
                xw_h1 = dram.tile([N2, OUT], F32, addr_space=aspace)
                xw_half = [xw_h0, xw_h1]

            # ---- constants ----
            w0 = consts.tile([128, OUT], F32)
            nc.sync.dma_start(w0[:], Wd[0:128, :])
            w1 = consts.tile([128, OUT], F32)
            nc.sync.dma_start(w1[:], Wd[128:256, :])
            idx_sb = consts.tile([128, TOTS // 16], I16)
            nc.sync.dma_start(idx_sb[:], IDXd[:])
            dest_sb = consts.tile([128, NINST], F32)
            nc.sync.dma_start(dest_sb[:], DESTd[:])
            val_sb = consts.tile([128, NINST], F32)
            nc.sync.dma_start(val_sb[:], VALd[:])
            nreal_sb = consts.tile([1, NCELL], I32)
            nc.sync.dma_start(nreal_sb[:], NREALd[:])
            iota_i = consts.tile([128, 128], I32)
            nc.gpsimd.iota(iota_i[:], pattern=[[1, 128]], base=0,
                           channel_multiplier=0)
            iota_f = consts.tile([128, 128], F32)
            nc.vector.tensor_copy(iota_f[:], iota_i[:])

            # ---- phase 1: xw_local = X_shard @ W  (XT is host-transposed) ----
            PANEL = 1024
            for p0 in range(0, R, PANEL):
                pw = min(PANEL, R - p0)
                xt0 = xtp.tile([128, PANEL], F32, tag="xt0")
                xt1 = xtp.tile([128, PANEL], F32, tag="xt1")
                nc.sync.dma_start(xt0[:, :pw], XTd[0:128, p0:p0 + pw])
                nc.sync.dma_start(xt1[:, :pw], XTd[128:256, p0:p0 + pw])
                for t0 in range(0, pw, 128):
                    cnt = min(128, pw - t0)
                    ps = psum_xw.tile([128, OUT], F32)
                    nc.tensor.matmul(ps[:cnt, :], lhsT=xt0[:, t0:t0 + cnt],
                                     rhs=w0[:], start=True, stop=False)
                    nc.tensor.matmul(ps[:cnt, :], lhsT=xt1[:, t0:t0 + cnt],
                                     rhs=w1[:], start=False, stop=True)
                    stg = xwstage.tile([128, OUT], F32)
                    nc.scalar.copy(stg[:cnt, :], ps[:cnt, :])
                    # write to the half-shard tiles (may straddle R2)
                    lo, hi = p0 + t0, p0 + t0 + cnt
                    if lo < R2:
                        c0 = min(hi, R2) - lo
                        nc.sync.dma_start(xw_lo0[lo:lo + c0, :], stg[:c0, :])
                    if hi > R2:
                        s0 = max(lo, R2)
                        nc.sync.dma_start(xw_lo1[s0 - R2:hi - R2, :],
                                          stg[s0 - lo:cnt, :])

            # ---- phase 2: AllGather XW shards (two halves, pipelined) ----
            if not single_core:
                for h, (src, dst) in enumerate([(xw_lo0, xw_half[0]),
                                                (xw_lo1, xw_half[1])]):
                    if xw_mode == "fill":
                        for q in range(cfg.P):
                            nc.sync.dma_start(dst[q * R2:(q + 1) * R2, :],
                                              src[:])
                    else:
                        nc.gpsimd.collective_compute(
                            "AllGather", mybir.AluOpType.bypass,
                            replica_groups=[list(range(cfg.P))],
                            ins=[src[:]], outs=[dst[:]],
                        )

            # ---- phase 3: per-group gather + one-hot matmul aggregation ----
            regs = [stack.enter_context(nc.gpsimd.register(name=f"nreal_r{i}"))
                    for i in range(2)]
            ci = 0
            for g in range(NG):
                gts = []
                for b in range(NBLK):
                    nch = NCHUNK[b]
                    gt = gpool.tile([128, nch * 128], F32, tag=f"g{b}")
                    off16 = (g * S_TOT + int(boff[b])) // 16
                    if use_gather:
                        if g < 2:
                            # first use of each pool slot: clear so skipped
                            # (idx=-1) slots hold finite values, not NaN junk
                            nc.vector.memset(gt[:], 0.0)
                        cell = g * NBLK + b
                        r = regs[ci % 2]
                        ci += 1
                        nc.gpsimd.reg_load(r, nreal_sb[0:1, cell:cell + 1])
                        base = (b % NBH) * BLK
                        nc.gpsimd.dma_gather(
                            gt[:].rearrange("p (c e) -> p c e", e=128),
                            xw_half[b // NBH][base:base + blk_rows[b], :],
                            idx_sb[:, off16:off16 + (nch * 128) // 16],
                            num_idxs=nch * 128,
                            num_idxs_reg=r,
                            elem_size=OUT,
                            single_packet=False,
                        )
                    else:
                        src = xw_half[b // NBH][0:nch * 128, :]
                        nc.sync.dma_start(
                            gt[:], src.rearrange("(p c) e -> p (c e)", p=128))
                    gts.append(gt)
                jmax = min(GRP, NW - g * GRP)
                for j in range(jmax):
                    w_global = g * GRP + j
                    row0 = w_global * WIN
                    cnt = min(WIN, R - row0)
                    insts = win_insts[(g, j)]
                    ot = outp.tile([128, OUT], F32)
                    if not insts:
                        nc.vector.memset(ot[:cnt, :], 0.0)
                    else:
                        ps = psum_e.tile([128, OUT], F32)
                        n = len(insts)
                        for k, (b, ch, inst) in enumerate(insts):
                            st = stp.tile([128, 128], F32)
                            nc.vector.tensor_scalar(
                                out=st[:], in0=iota_f[:],
                                scalar1=dest_sb[:, inst:inst + 1],
                                scalar2=val_sb[:, inst:inst + 1],
                                op0=mybir.AluOpType.is_equal,
                                op1=mybir.AluOpType.mult,
                            )
                            nc.tensor.matmul(
                                ps[:], lhsT=st[:],
                                rhs=gts[b][:, ch * 128:(ch + 1) * 128],
                                start=(k == 0), stop=(k == n - 1),
                            )
                        nc.scalar.copy(ot[:cnt, :], ps[:cnt, :])
                    nc.sync.dma_start(OUTd[row0:row0 + cnt, :], ot[:cnt, :])

    nc.compile()
    return nc


def _make_in_maps(cfg, X, W, per_core):
    X = np.ascontiguousarray(np.asarray(X, dtype=np.float32))
    W = np.ascontiguousarray(np.asarray(W, dtype=np.float32))
    in_maps = []
    for p in range(cfg.P):
        xt = np.ascontiguousarray(X[p * cfg.R:(p + 1) * cfg.R].T)
        in_maps.append({
            "XT": xt,
            "W": W,
            "IDX": per_core[p]["idx"],
            "DEST": per_core[p]["dest"],
            "VAL": per_core[p]["val"],
            "NREAL": per_core[p]["nreal"],
        })
    return in_maps


def prepare(cfg, X, W, edge_row, edge_col, edge_val):
    """Plan + build + compile; returns (nc, in_maps)."""
    edge_row = np.asarray(edge_row)
    edge_col = np.asarray(edge_col)
    edge_val = np.asarray(edge_val)
    static, per_core = _plan(cfg, edge_row, edge_col, edge_val)
    nc = _build(cfg, static)
    in_maps = _make_in_maps(cfg, X, W, per_core)
    return nc, in_maps


def execute(cfg, nc, in_maps, trace=False):
    res = run_bass_kernel_spmd(nc, in_maps, list(range(cfg.P)), trace=trace)
    out = np.concatenate([res.results[p]["OUT"] for p in range(cfg.P)], axis=0)
    return out.astype(np.float32), res


def kernel(X, W, edge_row, edge_col, edge_val):
    nc, in_maps = prepare(CFG, X, W, edge_row, edge_col, edge_val)
    out, _ = execute(CFG, nc, in_maps, trace=False)
    return out


# revision 32
# speedup vs baseline: 1.0529x; 1.0529x over previous
"""GCNConv (out = segment_sum(val * (X@W)[col], row)) on 8 TRN2 NeuronCores.

Sharding: output rows (nodes) are sharded across the 8 cores (12500 rows
each); W is replicated.  Each core computes its shard of XW = X @ W, the
shards are AllGathered into a full XW table in every core's DRAM, and each
core then aggregates only its own output rows:

    out[r] = sum over edges (r, c) of  val * XW[c]

The aggregation is implemented as dma_gather of XW rows (the source nodes of
the core's edges, pre-sorted on the host by destination window / source
block) followed by one-hot-matrix matmuls accumulating 128-destination-row
windows in PSUM:  out_win += S @ G  where S[d, e] = val_e * [dest_e == d]
is built on the vector engine from a single fused tensor_scalar
(iota == dest) * val, and G holds the gathered XW rows (one edge per
partition).

Host-side preprocessing (inside kernel()) only shards / sorts / pads the
edge list with numpy; all FLOPs and all memory-heavy work run on device.
"""

from contextlib import ExitStack

import numpy as np

import concourse.bass as bass
import concourse.mybir as mybir
from concourse import bacc, tile
from concourse.bass_utils import run_bass_kernel_spmd

F32 = mybir.dt.float32
F16 = mybir.dt.float16
I16 = mybir.dt.int16
I32 = mybir.dt.int32


class Cfg:
    def __init__(self, n_nodes=100000, in_dim=256, out_dim=128, ncores=8,
                 win=128, grp=4, blk=32768, table_fp16=False):
        self.N = n_nodes
        self.IN = in_dim
        self.OUT = out_dim
        self.P = ncores
        self.R = n_nodes // ncores          # rows (nodes) per core
        self.WIN = win                      # destination window (PSUM partitions)
        self.GRP = grp                      # windows per gather group
        self.BLK = blk                      # gather-table block (int16 index limit)
        self.NW = -(-self.R // win)         # windows per core
        self.NG = -(-self.NW // grp)        # groups per core
        # The XW table is AllGathered in two halves (so block-0/1 gathers can
        # start while the second AllGather is in flight).  Table layout is
        # "half-major": half h holds rows (p, r) for r in [h*R/2, (h+1)*R/2)
        # of every rank p, concatenated by rank.
        self.N2 = self.N // 2               # rows per half
        self.R2 = self.R // 2
        self.NBH = -(-self.N2 // blk)       # blocks per half
        self.NBLK = 2 * self.NBH
        # fp16 XW table: halves gather DMA traffic and enables PE fast
        # weight load + DVE 2-byte perf modes.  Costs ~3e-4 relative error
        # (vs ~1.4e-7 all-fp32), so off by default.
        self.table_fp16 = table_fp16
        assert n_nodes % ncores == 0 and self.R % 2 == 0
        assert blk <= 32768

    def remap(self, col):
        """Node id -> position in the half-major AllGather table layout."""
        p, r = np.divmod(col, self.R)
        lo = r < self.R2
        return np.where(lo, p * self.R2 + r,
                        self.N2 + p * self.R2 + (r - self.R2))


CFG = Cfg()


def _plan(cfg, edge_row, edge_col, edge_val):
    """Partition/sort/pad the edge list per core. Returns (static, per_core).

    Static structure (identical for all cores, required for SPMD):
      - MAXS[b]: padded slot count of each (group, block) gather cell
      - instance list: (group, window-in-group, block, chunk) matmul chunks
    Per core:
      - IDX  [128, TOTS//16] int16: gather indices (16-part wrap, replicated x8)
      - DEST [128, NINST] f32: per-chunk-instance local dest row (-1 = inactive)
      - VAL  [128, NINST] f32: per-chunk-instance edge weight (0 = inactive)
    """
    P, R, WIN, GRP, BLK, NBLK = cfg.P, cfg.R, cfg.WIN, cfg.GRP, cfg.BLK, cfg.NBLK
    NW, NG = cfg.NW, cfg.NG
    NCELL = NG * NBLK

    cores = []
    for p in range(P):
        s = np.searchsorted(edge_row, p * R, side="left")
        e = np.searchsorted(edge_row, (p + 1) * R, side="left")
        r = edge_row[s:e].astype(np.int64) - p * R
        c = edge_col[s:e].astype(np.int64)
        v = edge_val[s:e].astype(np.float32)
        w = r // WIN
        g = w // GRP
        pos = cfg.remap(c)                 # position in half-major table
        half = pos // cfg.N2
        off = pos - half * cfg.N2
        b = half * cfg.NBH + off // BLK
        c = off % BLK                      # index within block
        order = np.lexsort((w, b, g))
        r, c, v, w, g, b = (a[order] for a in (r, c, v, w, g, b))
        cell = g * NBLK + b
        counts = np.bincount(cell, minlength=NCELL)
        cstart = np.concatenate([[0], np.cumsum(counts)[:-1]])
        pos = np.arange(len(r)) - cstart[cell]
        j = w - g * GRP
        cnt_cwj = np.bincount(cell * GRP + j, minlength=NCELL * GRP)
        cnt_cwj = cnt_cwj.reshape(NCELL, GRP)
        cores.append(dict(r=r, c=c, v=v, w=w, g=g, b=b, cell=cell, pos=pos,
                          counts=counts, cnt_cwj=cnt_cwj))

    all_counts = np.stack([cc["counts"] for cc in cores])      # [P, NCELL]
    per_gb = all_counts.reshape(P, NG, NBLK)
    maxs = per_gb.max(axis=(0, 1))                             # [NBLK]
    MAXS = np.maximum(128, ((maxs + 127) // 128) * 128).astype(np.int64)
    S_TOT = int(MAXS.sum())
    boff = np.concatenate([[0], np.cumsum(MAXS)[:-1]]).astype(np.int64)
    TOTS = NG * S_TOT

    # instance enumeration (static): for each (g, j): the (b, chunk) matmuls
    inst_list = []
    win_insts = {}
    maxch = int(MAXS.max()) // 128
    L = -np.ones((NCELL, maxch, GRP), np.int64)
    for g in range(NG):
        jmax = min(GRP, NW - g * GRP)
        for j in range(jmax):
            lst = []
            for b in range(NBLK):
                cell = g * NBLK + b
                c0, c1 = 10 ** 9, -1
                for cc in cores:
                    cnts = cc["cnt_cwj"][cell]
                    st = int(cnts[:j].sum())
                    en = st + int(cnts[j])
                    if en > st:
                        c0 = min(c0, st // 128)
                        c1 = max(c1, -(-en // 128))
                if c1 < 0:
                    continue
                for ch in range(c0, c1):
                    inst_id = len(inst_list)
                    inst_list.append((g, j, b, ch))
                    L[cell, ch, j] = inst_id
                    lst.append((b, ch, inst_id))
            win_insts[(g, j)] = lst
    NINST = len(inst_list)

    per_core = []
    for cc in cores:
        dest = np.full((128, max(NINST, 1)), -1.0, np.float32)
        val = np.zeros((128, max(NINST, 1)), np.float32)
        # -1 = "skip" (no DMA); real indices first in each cell, -1 tail.
        idx = np.full(TOTS, -1, np.int16)
        slot = cc["g"] * S_TOT + boff[cc["b"]] + cc["pos"]
        idx[slot] = cc["c"].astype(np.int16)   # already block-local
        # per-cell live count for num_idxs_reg; keep a >=16 floor (the gather
        # ucode wraps indices over 16 channels) by padding tiny cells with
        # real row-0 reads.
        nreal = cc["counts"].astype(np.int64).copy()
        for cell_id in np.nonzero(nreal < 16)[0]:
            g_, b_ = divmod(cell_id, NBLK)
            base = g_ * S_TOT + boff[b_]
            idx[base + nreal[cell_id]:base + 16] = 0
            nreal[cell_id] = 16
        chunk = cc["pos"] // 128
        jj = cc["w"] - cc["g"] * GRP
        inst = L[cc["cell"], chunk, jj]
        assert (inst >= 0).all()
        part = cc["pos"] % 128
        dest[part, inst] = (cc["r"] % WIN).astype(np.float32)
        val[part, inst] = cc["v"]
        idx128 = np.tile(idx.reshape(-1, 16).T, (8, 1))        # [128, TOTS//16]
        per_core.append(dict(idx=np.ascontiguousarray(idx128),
                             dest=dest, val=val,
                             nreal=nreal.astype(np.int32).reshape(1, -1)))

    static = dict(MAXS=MAXS, S_TOT=S_TOT, TOTS=TOTS, boff=boff,
                  NINST=max(NINST, 1), win_insts=win_insts)
    return static, per_core


def _build(cfg, static, single_core=False, xw_mode="ag", use_gather=True):
    """Trace + schedule + compile the SPMD Bass program (one NEFF, 8 cores).

    single_core=True builds a collective-free variant for TimelineSim cost
    modeling: the gather table is an ExternalInput instead of the AllGather
    output (the AllGather itself costs ~35us extra; see collectives.md).

    xw_mode: "ag" (shard + AllGather), "fill" (no collective; xw_full filled
    with 8 DMA copies of the local shard -- wrong data, crash-bisect only),
    "local" (AllGather with Local instead of Shared scratchpad).
    use_gather=False replaces dma_gather with contiguous DMA reads of the
    same size (wrong data, crash-bisect only).
    """
    R, IN, OUT, WIN, GRP, BLK, NBLK = (cfg.R, cfg.IN, cfg.OUT, cfg.WIN,
                                       cfg.GRP, cfg.BLK, cfg.NBLK)
    NW, NG, N = cfg.NW, cfg.NG, cfg.N
    MAXS, S_TOT, TOTS = static["MAXS"], static["S_TOT"], static["TOTS"]
    boff, NINST, win_insts = static["boff"], static["NINST"], static["win_insts"]
    NCHUNK = [int(m) // 128 for m in MAXS]

    N2, R2, NBH = cfg.N2, cfg.R2, cfg.NBH
    TDT = F16 if cfg.table_fp16 else F32

    nc = bacc.Bacc("TRN2", target_bir_lowering=False, debug=False,
                   num_devices=1 if single_core else cfg.P)
    XWFULLd = None
    if single_core:
        XWFULLd = nc.dram_tensor("XWFULL", [N, OUT], TDT,
                                 kind="ExternalInput").ap()
    XTd = nc.dram_tensor("XT", [IN, R], F32, kind="ExternalInput").ap()
    Wd = nc.dram_tensor("W", [IN, OUT], F32, kind="ExternalInput").ap()
    IDXd = nc.dram_tensor("IDX", [128, TOTS // 16], I16, kind="ExternalInput").ap()
    DESTd = nc.dram_tensor("DEST", [128, NINST], F32, kind="ExternalInput").ap()
    VALd = nc.dram_tensor("VAL", [128, NINST], F32, kind="ExternalInput").ap()
    NCELL = NG * NBLK
    NREALd = nc.dram_tensor("NREAL", [1, NCELL], I32, kind="ExternalInput").ap()
    OUTd = nc.dram_tensor("OUT", [R, OUT], F32, kind="ExternalOutput").ap()

    blk_rows = [min(BLK, N2 - (b % NBH) * BLK) for b in range(NBLK)]

    with tile.TileContext(nc) as tc:
        with (
            ExitStack() as stack,
            tc.tile_pool(name="dram", bufs=1, space="DRAM") as dram,
            tc.tile_pool(name="consts", bufs=1) as consts,
            tc.tile_pool(name="xtp", bufs=2) as xtp,
            tc.tile_pool(name="xwstage", bufs=3) as xwstage,
            tc.tile_pool(name="gpool", bufs=2) as gpool,
            tc.tile_pool(name="stp", bufs=4) as stp,
            tc.tile_pool(name="outp", bufs=4) as outp,
            tc.tile_pool(name="psum_xw", bufs=2, space="PSUM") as psum_xw,
            tc.tile_pool(name="psum_e", bufs=4, space="PSUM") as psum_e,
        ):
            xw_lo0 = dram.tile([R2, OUT], F32)
            xw_lo1 = dram.tile([R2, OUT], F32)
            if single_core:
                xw_half = [XWFULLd[0:N2, :], XWFULLd[N2:N, :]]
            else:
                aspace = "Shared" if xw_mode == "ag" else "Local"
                xw_h0 = dram.tile([N2, OUT], F32, addr_space=aspace)
                xw_h1 = dram.tile([N2, OUT], F32, addr_space=aspace)
                xw_half = [xw_h0, xw_h1]

            # ---- constants ----
            w0 = consts.tile([128, OUT], F32)
            nc.sync.dma_start(w0[:], Wd[0:128, :])
            w1 = consts.tile([128, OUT], F32)
            nc.sync.dma_start(w1[:], Wd[128:256, :])
            idx_sb = consts.tile([128, TOTS // 16], I16)
            nc.sync.dma_start(idx_sb[:], IDXd[:])
            dest_sb = consts.tile([128, NINST], F32)
            nc.sync.dma_start(dest_sb[:], DESTd[:])
            val_sb = consts.tile([128, NINST], F32)
            nc.sync.dma_start(val_sb[:], VALd[:])
            nreal_sb = consts.tile([1, NCELL], I32)
            nc.sync.dma_start(nreal_sb[:], NREALd[:])
            iota_i = consts.tile([128, 128], I32)
            nc.gpsimd.iota(iota_i[:], pattern=[[1, 128]], base=0,
                           channel_multiplier=0)
            iota_f = consts.tile([128, 128], F32)
            nc.vector.tensor_copy(iota_f[:], iota_i[:])

            # ---- phase 1: xw_local = X_shard @ W  (XT is host-transposed) ----
            PANEL = 1024
            for p0 in range(0, R, PANEL):
                pw = min(PANEL, R - p0)
                xt0 = xtp.tile([128, PANEL], F32, tag="xt0")
                xt1 = xtp.tile([128, PANEL], F32, tag="xt1")
                nc.sync.dma_start(xt0[:, :pw], XTd[0:128, p0:p0 + pw])
                nc.sync.dma_start(xt1[:, :pw], XTd[128:256, p0:p0 + pw])
                for t0 in range(0, pw, 128):
                    cnt = min(128, pw - t0)
                    ps = psum_xw.tile([128, OUT], F32)
                    nc.tensor.matmul(ps[:cnt, :], lhsT=xt0[:, t0:t0 + cnt],
                                     rhs=w0[:], start=True, stop=False)
                    nc.tensor.matmul(ps[:cnt, :], lhsT=xt1[:, t0:t0 + cnt],
                                     rhs=w1[:], start=False, stop=True)
                    stg = xwstage.tile([128, OUT], F32)
                    nc.scalar.copy(stg[:cnt, :], ps[:cnt, :])
                    # write to the half-shard tiles (may straddle R2)
                    lo, hi = p0 + t0, p0 + t0 + cnt
                    if lo < R2:
                        c0 = min(hi, R2) - lo
                        nc.sync.dma_start(xw_lo0[lo:lo + c0, :], stg[:c0, :])
                    if hi > R2:
                        s0 = max(lo, R2)
                        nc.sync.dma_start(xw_lo1[s0 - R2:hi - R2, :],
                                          stg[s0 - lo:cnt, :])

            # ---- phase 2: AllGather XW shards (two halves, pipelined) ----
            if not single_core:
                for h, (src, dst) in enumerate([(xw_lo0, xw_half[0]),
                                                (xw_lo1, xw_half[1])]):
                    if xw_mode == "fill":
                        for q in range(cfg.P):
                            nc.sync.dma_start(dst[q * R2:(q + 1) * R2, :],
                                              src[:])
                    else:
                        nc.gpsimd.collective_compute(
                            "AllGather", mybir.AluOpType.bypass,
                            replica_groups=[list(range(cfg.P))],
                            ins=[src[:]], outs=[dst[:]],
                        )

            # ---- phase 3: per-group gather + one-hot matmul aggregation ----
            regs = [stack.enter_context(nc.gpsimd.register(name=f"nreal_r{i}"))
                    for i in range(2)]
            ci = 0
            for g in range(NG):
                gts = []
                for b in range(NBLK):
                    nch = NCHUNK[b]
                    gt = gpool.tile([128, nch * 128], F32, tag=f"g{b}")
                    off16 = (g * S_TOT + int(boff[b])) // 16
                    if use_gather:
                        if g < 2:
                            # first use of each pool slot: clear so skipped
                            # (idx=-1) slots hold finite values, not NaN junk
                            nc.vector.memset(gt[:], 0.0)
                        cell = g * NBLK + b
                        r = regs[ci % 2]
                        ci += 1
                        nc.gpsimd.reg_load(r, nreal_sb[0:1, cell:cell + 1])
                        base = (b % NBH) * BLK
                        nc.gpsimd.dma_gather(
                            gt[:].rearrange("p (c e) -> p c e", e=128),
                            xw_half[b // NBH][base:base + blk_rows[b], :],
                            idx_sb[:, off16:off16 + (nch * 128) // 16],
                            num_idxs=nch * 128,
                            num_idxs_reg=r,
                            elem_size=OUT,
                            single_packet=False,
                        )
                    else:
                        src = xw_half[b // NBH][0:nch * 128, :]
                        nc.sync.dma_start(
                            gt[:], src.rearrange("(p c) e -> p (c e)", p=128))
                    gts.append(gt)
                jmax = min(GRP, NW - g * GRP)
                for j in range(jmax):
                    w_global = g * GRP + j
                    row0 = w_global * WIN
                    cnt = min(WIN, R - row0)
                    insts = win_insts[(g, j)]
                    ot = outp.tile([128, OUT], F32)
                    if not insts:
                        nc.vector.memset(ot[:cnt, :], 0.0)
                    else:
                        ps = psum_e.tile([128, OUT], F32)
                        n = len(insts)
                        for k, (b, ch, inst) in enumerate(insts):
                            st = stp.tile([128, 128], F32)
                            nc.vector.tensor_scalar(
                                out=st[:], in0=iota_f[:],
                                scalar1=dest_sb[:, inst:inst + 1],
                                scalar2=val_sb[:, inst:inst + 1],
                                op0=mybir.AluOpType.is_equal,
                                op1=mybir.AluOpType.mult,
                            )
                            nc.tensor.matmul(
                                ps[:], lhsT=st[:],
                                rhs=gts[b][:, ch * 128:(ch + 1) * 128],
                                start=(k == 0), stop=(k == n - 1),
                            )
                        nc.scalar.copy(ot[:cnt, :], ps[:cnt, :])
                    nc.sync.dma_start(OUTd[row0:row0 + cnt, :], ot[:cnt, :])

    nc.compile()
    return nc


def _make_in_maps(cfg, X, W, per_core):
    X = np.ascontiguousarray(np.asarray(X, dtype=np.float32))
    W = np.ascontiguousarray(np.asarray(W, dtype=np.float32))
    in_maps = []
    for p in range(cfg.P):
        xt = np.ascontiguousarray(X[p * cfg.R:(p + 1) * cfg.R].T)
        in_maps.append({
            "XT": xt,
            "W": W,
            "IDX": per_core[p]["idx"],
            "DEST": per_core[p]["dest"],
            "VAL": per_core[p]["val"],
            "NREAL": per_core[p]["nreal"],
        })
    return in_maps


def prepare(cfg, X, W, edge_row, edge_col, edge_val):
    """Plan + build + compile; returns (nc, in_maps)."""
    edge_row = np.asarray(edge_row)
    edge_col = np.asarray(edge_col)
    edge_val = np.asarray(edge_val)
    static, per_core = _plan(cfg, edge_row, edge_col, edge_val)
    nc = _build(cfg, static)
    in_maps = _make_in_maps(cfg, X, W, per_core)
    return nc, in_maps


def execute(cfg, nc, in_maps, trace=False):
    res = run_bass_kernel_spmd(nc, in_maps, list(range(cfg.P)), trace=trace)
    out = np.concatenate([res.results[p]["OUT"] for p in range(cfg.P)], axis=0)
    return out.astype(np.float32), res


def kernel(X, W, edge_row, edge_col, edge_val):
    nc, in_maps = prepare(CFG, X, W, edge_row, edge_col, edge_val)
    out, _ = execute(CFG, nc, in_maps, trace=False)
    return out
